# revision 4
# baseline (speedup 1.0000x reference)
"""Distance-scorer Bass kernel for 8 Trainium2 NeuronCores — v2.

Math: score[b,k] = W2 . relu(W1[bin,:] + x*W1[50] + y*W1[51]
                             + ego[b] @ W1[52:56] + b1) + b2
with s = x^2 + y^2, d = sqrt(s), v = fp32(fp32(d/50)*50), bin = clip(floor(v), 0, 49).

v2 design (vs v1 baseline at ~670 us):
  - Per-element bin KEY k = min(floor(v), 49) computed arithmetically
    (Act sqrt + DVE divide/mult/mod/floor chain) instead of 50 fp32 threshold
    compares on replicated s. Exact given exact fp32 sqrt/divide.
  - Keys (bf16, exact small integers) bounce through DRAM and are replicated
    across 98 partitions by 2 DMAs per 32-row group (26x less replication
    bytes than fp32 s replication would need).
  - Staircase stair[j] = (k >= j), j=1..49 per element half, via in-place DVE
    is_ge (bf16, 2x DVE perf mode) inside a 32-row mega-tile.
  - mm1 is ONE bf16 matmul per row: contraction 107 = 49 stairs x 2 halves
    + x,y rows + 4 ego rows + ones row. Ego and the b1+T[0] constant ride the
    contraction (stride-0 broadcast DMAs), removing the per-row activation
    bias so relu can batch 2 PSUM banks per instruction.
  - relu on Act (bias-free), scores = bf16 mm2 (+-1 weights) into 4-row PSUM
    groups (tile_position col 0/32/64/96), b2 added during the PSUM->SBUF
    copy into a 32-row staging tile, 4 DMAs out per 32 rows.
"""

import numpy as np

N_CORES = 8
B, K = 2048, 1024
NB = 50
D = 64
BS = B // N_CORES  # 256 rows/core
F = K // 2  # 512; elements (c, c+512) of a row share a column
NTH = NB - 1  # 49 staircase rows per element half (j = 1..49)
KC = 2 * NTH + 4 + 5  # 107 = stairs A/B + xA,xB,yA,yB + 4 ego + ones
RPB = 64  # rows per block
RPG = 32  # rows per mega-tile group
GPB = RPB // RPG  # groups per block
RELU_DVE_EVERY = 2  # every Nth 2-row relu unit runs on DVE instead of Act
COPY_ACT_EVERY = 2  # every Nth score copy runs on Act instead of DVE
MEGA_BUFS = 2  # mega-tile double/triple buffering
EDGE_SPLIT = False  # split first/last block into 16-row mini-groups
XY_Q = "sync"  # queue for the x/y mega-row DMAs: sync | gpsimd | scalar


def _consts(W1, b1, W2, b2):
    import ml_dtypes

    W1 = np.asarray(W1, np.float32)
    b1 = np.asarray(b1, np.float32)
    W2 = np.asarray(W2, np.float32)
    absw2 = np.abs(W2[:, 0])
    sgn = np.where(W2[:, 0] < 0, -1.0, 1.0).astype(np.float32)

    Tt = W1[:NB] * absw2[None, :]  # (50, 64)
    dT = (Tt[1:] - Tt[:-1]).astype(np.float32)  # (49, 64), rows j=1..49
    wx = W1[50] * absw2
    wy = W1[51] * absw2
    wego = W1[52:56] * absw2[None, :]  # (4, 64)
    wc = (b1 + W1[0]) * absw2  # b1 + T[0] folded

    bf = ml_dtypes.bfloat16
    smm1 = np.zeros((KC, 128), bf)
    smm1[0:NTH, 0:64] = dT.astype(bf)
    smm1[NTH : 2 * NTH, 64:128] = dT.astype(bf)
    smm1[98, 0:64] = wx.astype(bf)
    smm1[99, 64:128] = wx.astype(bf)
    smm1[100, 0:64] = wy.astype(bf)
    smm1[101, 64:128] = wy.astype(bf)
    for i in range(4):
        smm1[102 + i, 0:64] = wego[i].astype(bf)
        smm1[102 + i, 64:128] = wego[i].astype(bf)
    smm1[106, 0:64] = wc.astype(bf)
    smm1[106, 64:128] = wc.astype(bf)

    # smm2e: first mm2 of each PSUM group writes the FULL bank (cols 2:128
    # zero) so the later full-tile copy never reads uninitialized PSUM.
    smm2 = np.zeros((128, 2), bf)
    smm2[0:64, 0] = sgn.astype(bf)
    smm2[64:128, 1] = sgn.astype(bf)
    smm2e = np.zeros((128, 128), bf)
    smm2e[:, 0:2] = smm2

    thr = np.zeros((2 * NTH, 1), np.float32)
    thr[0:NTH, 0] = np.arange(1, NB, dtype=np.float32)
    thr[NTH : 2 * NTH, 0] = np.arange(1, NB, dtype=np.float32)

    b2col = np.full((128, 1), float(np.asarray(b2, np.float32).reshape(-1)[0]),
                    np.float32)
    return dict(smm1=smm1, smm2=smm2, smm2e=smm2e, thr=thr, b2col=b2col)


def _ego_rows(ego):
    """Stage ego (+ a ones row) replicated along columns: [B, 5, F] bf16."""
    import ml_dtypes

    e5 = np.ones((ego.shape[0], 5), np.float32)
    e5[:, 0:4] = ego
    e5b = e5.astype(ml_dtypes.bfloat16)
    return np.ascontiguousarray(np.broadcast_to(e5b[:, :, None],
                                                (ego.shape[0], 5, F)))


def _build():
    import concourse.bass as bass
    import concourse.mybir as mybir
    from concourse import bacc
    from concourse.tile import TileContext

    f32 = mybir.dt.float32
    bf16 = mybir.dt.bfloat16
    Alu = mybir.AluOpType
    Act = mybir.ActivationFunctionType

    nc = bacc.Bacc("TRN2", target_bir_lowering=False, debug=False,
                   num_devices=N_CORES)

    gpx_d = nc.declare_dram_parameter("gpx", [BS, K], f32, isOutput=False)
    gpy_d = nc.declare_dram_parameter("gpy", [BS, K], f32, isOutput=False)
    egor_d = nc.declare_dram_parameter("egor", [BS, 5, F], bf16, isOutput=False)
    smm1_d = nc.declare_dram_parameter("smm1", [KC, 128], bf16, isOutput=False)
    smm2_d = nc.declare_dram_parameter("smm2", [128, 2], bf16, isOutput=False)
    smm2e_d = nc.declare_dram_parameter("smm2e", [128, 128], bf16, isOutput=False)
    thr_d = nc.declare_dram_parameter("thr", [2 * NTH, 1], f32, isOutput=False)
    b2_d = nc.declare_dram_parameter("b2c", [128, 1], f32, isOutput=False)
    sc_d = nc.declare_dram_parameter("scores", [BS, K], f32, isOutput=True)

    with TileContext(nc) as tc:
        with (
            tc.tile_pool(name="consts", bufs=1) as cpool,
            tc.tile_pool(name="work", bufs=2) as wpool,
            tc.tile_pool(name="mega", bufs=MEGA_BUFS) as spool,
            tc.tile_pool(name="mega16", bufs=2) as spool16,
            tc.tile_pool(name="hrelu", bufs=6) as hpool,
            tc.tile_pool(name="scr", bufs=2) as scpool,
            tc.tile_pool(name="scr16", bufs=2) as scpool16,
            tc.tile_pool(name="edram", bufs=1, space="DRAM") as epool,
            tc.tile_pool(name="sdram", bufs=2, space="DRAM") as dpool,
            tc.tile_pool(name="p1", bufs=3, space="PSUM") as p1pool,
            tc.tile_pool(name="p2", bufs=2, space="PSUM") as p2pool,
        ):
            smm1_s = cpool.tile([KC, 128], bf16, tag="smm1")
            smm2_s = cpool.tile([128, 2], bf16, tag="smm2")
            smm2e_s = cpool.tile([128, 128], bf16, tag="smm2e")
            thr_s = cpool.tile([2 * NTH, 1], f32, tag="thr")
            b2_s = cpool.tile([128, 1], f32, tag="b2c")
            nc.sync.dma_start(out=smm1_s[:], in_=smm1_d[:])
            nc.sync.dma_start(out=smm2_s[:], in_=smm2_d[:])
            nc.sync.dma_start(out=smm2e_s[:], in_=smm2e_d[:])
            nc.sync.dma_start(out=thr_s[:], in_=thr_d[:])
            nc.sync.dma_start(out=b2_s[:], in_=b2_d[:])

            for blk in range(BS // RPB):
                r0 = blk * RPB
                gpxp = wpool.tile([128, F], f32, tag="gpxp")
                gpyp = wpool.tile([128, F], f32, tag="gpyp")
                nc.gpsimd.dma_start(
                    out=gpxp[:],
                    in_=gpx_d[r0 : r0 + RPB].rearrange("b (h k) -> (b h) k", h=2))
                nc.gpsimd.dma_start(
                    out=gpyp[:],
                    in_=gpy_d[r0 : r0 + RPB].rearrange("b (h k) -> (b h) k", h=2))

                xx = wpool.tile([128, F], f32, tag="xx")
                ss = wpool.tile([128, F], f32, tag="ss")
                nc.vector.tensor_tensor(out=xx[:], in0=gpxp[:], in1=gpxp[:],
                                        op=Alu.mult)
                nc.vector.tensor_tensor(out=ss[:], in0=gpyp[:], in1=gpyp[:],
                                        op=Alu.mult)
                nc.vector.tensor_tensor(out=ss[:], in0=ss[:], in1=xx[:],
                                        op=Alu.add)
                # d = sqrt(s); v = (d*(1/50))*50;
                # floor(v) = rne(v) - (rne(v) > v), rne via the +-1.5*2^23
                # magic constant (valid-ISA ops only: divide/mod are not
                # legal DVE tensor_scalar ops on hardware).
                MAGIC = 12582912.0  # 1.5 * 2^23
                dv = wpool.tile([128, F], f32, tag="dv")
                nc.scalar.activation(dv[:], ss[:], Act.Sqrt)
                nc.vector.tensor_scalar(out=dv[:], in0=dv[:], scalar1=0.02,
                                        scalar2=50.0, op0=Alu.mult,
                                        op1=Alu.mult)
                mm = wpool.tile([128, F], f32, tag="mm")
                nc.vector.tensor_scalar(out=mm[:], in0=dv[:], scalar1=MAGIC,
                                        scalar2=-MAGIC, op0=Alu.add,
                                        op1=Alu.add)
                gg = wpool.tile([128, F], f32, tag="gg")
                nc.vector.tensor_tensor(out=gg[:], in0=mm[:], in1=dv[:],
                                        op=Alu.is_gt)
                nc.vector.tensor_tensor(out=mm[:], in0=mm[:], in1=gg[:],
                                        op=Alu.subtract)
                d16 = wpool.tile([128, F], bf16, tag="d16")
                nc.vector.tensor_scalar(out=d16[:], in0=mm[:], scalar1=49.0,
                                        scalar2=None, op0=Alu.min)
                # bf16 copies of x, y for mm1 xy contraction rows
                gxb = wpool.tile([128, F], bf16, tag="gxb")
                gyb = wpool.tile([128, F], bf16, tag="gyb")
                nc.vector.tensor_copy(out=gxb[:], in_=gpxp[:])
                nc.vector.tensor_copy(out=gyb[:], in_=gpyp[:])
                # bounce keys + bf16 xy through DRAM for group-level broadcast
                d16_d = dpool.tile([RPB, K], bf16, tag="d16d")
                xb_d = dpool.tile([RPB, K], bf16, tag="xbd")
                yb_d = dpool.tile([RPB, K], bf16, tag="ybd")
                for sb, dr in ((d16, d16_d), (gxb, xb_d), (gyb, yb_d)):
                    nc.scalar.dma_start(
                        out=dr[:].rearrange("b (h k) -> (b h) k", h=2),
                        in_=sb[:])

                if EDGE_SPLIT and blk == 0:
                    plan = [(0, 16), (16, 16), (32, 32)]
                elif EDGE_SPLIT and blk == BS // RPB - 1:
                    plan = [(0, 32), (32, 16), (48, 16)]
                else:
                    plan = [(0, RPG), (RPG, RPG)]
                for g0, rpg in plan:
                    gr0 = r0 + g0
                    CW = rpg * F
                    if rpg == RPG:
                        mega = spool.tile([KC, CW], bf16, tag="mega")
                    else:
                        mega = spool16.tile([KC, CW], bf16, tag="mega16")

                    # key replication: 49 partitions x 32 rows, A and B halves,
                    # split into 8-row chunks so transfers interleave finely
                    HR = 8
                    NCH = rpg // HR
                    for ch in range(NCH):
                        for half, prow in ((0, 0), (1, NTH)):
                            src = d16_d[g0 + ch * HR : g0 + (ch + 1) * HR]
                            src_a = bass.AP(
                                tensor=src.tensor, offset=src.offset + half * F,
                                ap=[[0, NTH], [K, HR], [1, F]])
                            nc.sync.dma_start(
                                out=mega[prow : prow + NTH,
                                         ch * HR * F : (ch + 1) * HR * F
                                         ].rearrange("p (r c) -> p r c", r=HR),
                                in_=src_a)
                    # x,y rows from DRAM bf16 scratch
                    for dr, prow in ((xb_d, 98), (yb_d, 100)):
                        src = dr[g0 : g0 + rpg]
                        src_a = bass.AP(
                            tensor=src.tensor, offset=src.offset,
                            ap=[[F, 2], [K, rpg], [1, F]])
                        getattr(nc, XY_Q).dma_start(
                            out=mega[prow : prow + 2, :].rearrange(
                                "p (r c) -> p r c", r=rpg),
                            in_=src_a)
                    # ego rows + ones row (pre-replicated along columns)
                    esrc = egor_d[gr0 : gr0 + rpg]
                    esrc_a = bass.AP(tensor=esrc.tensor, offset=esrc.offset,
                                     ap=[[F, 5], [5 * F, rpg], [1, F]])
                    nc.gpsimd.dma_start(
                        out=mega[102:107, :].rearrange("p (r c) -> p r c", r=rpg),
                        in_=esrc_a)

                    # staircase in place: is_ge per 8-row chunk
                    for ci in range(NCH):
                        sl = slice(ci * CW // NCH, (ci + 1) * CW // NCH)
                        nc.vector.tensor_scalar(
                            out=mega[0 : 2 * NTH, sl],
                            in0=mega[0 : 2 * NTH, sl],
                            scalar1=thr_s[:], scalar2=None, op0=Alu.is_ge)

                    if rpg == RPG:
                        scr = scpool.tile([128, rpg // 4 * F], f32, tag="scr")
                    else:
                        scr = scpool16.tile([128, rpg // 4 * F], f32,
                                            tag="scr16")
                    for pj in range(rpg // 4):
                        rr = gr0 + pj * 4
                        j0 = pj * 4
                        p1a = p1pool.tile([128, 2 * F], f32, tag="p1")
                        p1b = p1pool.tile([128, 2 * F], f32, tag="p1")
                        for jj in range(4):
                            pt = p1a if jj < 2 else p1b
                            nc.tensor.matmul(
                                pt[:, (jj % 2) * F : (jj % 2 + 1) * F],
                                lhsT=smm1_s[:],
                                rhs=mega[:, (j0 + jj) * F : (j0 + jj + 1) * F],
                                start=True, stop=True)
                        hra = hpool.tile([128, 2 * F], bf16, tag="hr")
                        hrb = hpool.tile([128, 2 * F], bf16, tag="hr")
                        nc.scalar.activation(hra[:], p1a[:], Act.Relu)
                        if pj % RELU_DVE_EVERY == RELU_DVE_EVERY - 1:
                            nc.vector.tensor_scalar(
                                out=hrb[:], in0=p1b[:], scalar1=0.0,
                                scalar2=None, op0=Alu.max)
                        else:
                            nc.scalar.activation(hrb[:], p1b[:], Act.Relu)
                        p2 = p2pool.tile([128, F], f32, tag="p2")
                        nc.tensor.matmul(p2[:], lhsT=smm2e_s[:],
                                         rhs=hra[:, 0:F], start=True, stop=True)
                        for jj in range(1, 4):
                            hx = hra if jj < 2 else hrb
                            nc.tensor.matmul(
                                p2[32 * jj : 32 * jj + 2, :],
                                lhsT=smm2_s[:],
                                rhs=hx[:, (jj % 2) * F : (jj % 2 + 1) * F],
                                start=True, stop=True,
                                tile_position=(0, 32 * jj))
                        # b2 add during PSUM -> SBUF staging copy
                        dstc = scr[:, pj * F : (pj + 1) * F]
                        if pj % COPY_ACT_EVERY == 0:
                            nc.scalar.activation(dstc, p2[:], Act.Identity,
                                                 bias=b2_s[:, 0:1])
                        else:
                            nc.vector.tensor_scalar(
                                out=dstc, in0=p2[:], scalar1=b2_s[:, 0:1],
                                scalar2=None, op0=Alu.add)
                    # 4 DMAs out per 32 rows: partition pair {32a, 32a+1}
                    # holds rows gr0 + a + 4t for t in 0..8
                    for a in range(4):
                        dst = sc_d[gr0 + a : gr0 + a + 1]
                        dst_a = bass.AP(
                            tensor=dst.tensor, offset=dst.offset,
                            ap=[[F, 2], [4 * K, rpg // 4], [1, F]])
                        nc.gpsimd.dma_start(
                            out=dst_a,
                            in_=scr[32 * a : 32 * a + 2, :].rearrange(
                                "p (t c) -> p t c", t=rpg // 4))

    nc.finalize()
    return nc


_CACHE = {}


def make_in_maps(goal_positions, ego_state, W1, b1, W2, b2):
    gp = np.asarray(goal_positions, np.float32)
    gpx = np.ascontiguousarray(gp[..., 0])
    gpy = np.ascontiguousarray(gp[..., 1])
    egor = _ego_rows(np.asarray(ego_state, np.float32))
    c = _consts(W1, b1, W2, b2)
    in_maps = []
    for i in range(N_CORES):
        in_maps.append({
            "gpx": gpx[i * BS : (i + 1) * BS],
            "gpy": gpy[i * BS : (i + 1) * BS],
            "egor": egor[i * BS : (i + 1) * BS],
            "smm1": c["smm1"], "smm2": c["smm2"], "smm2e": c["smm2e"],
            "thr": c["thr"], "b2c": c["b2col"],
        })
    return in_maps


def kernel(goal_positions, ego_state, W1, b1, W2, b2):
    from concourse.bass_utils import run_bass_kernel_spmd

    if "nc" not in _CACHE:
        _CACHE["nc"] = _build()
    nc = _CACHE["nc"]

    in_maps = make_in_maps(goal_positions, ego_state, W1, b1, W2, b2)
    res = run_bass_kernel_spmd(nc, in_maps, core_ids=list(range(N_CORES)))
    out = np.concatenate([res.results[i]["scores"] for i in range(N_CORES)],
                         axis=0)
    return out.astype(np.float32)


# revision 6
# speedup vs baseline: 1.0030x; 1.0030x over previous
"""Distance-scorer Bass kernel for 8 Trainium2 NeuronCores — v2.

Math: score[b,k] = W2 . relu(W1[bin,:] + x*W1[50] + y*W1[51]
                             + ego[b] @ W1[52:56] + b1) + b2
with s = x^2 + y^2, d = sqrt(s), v = fp32(fp32(d/50)*50), bin = clip(floor(v), 0, 49).

v2 design (vs v1 baseline at ~670 us; TimelineSim estimate ~165 us):
  - Per-element bin KEY k = min(floor(v), 49) computed arithmetically
    (Act sqrt + DVE divide/mult/mod/floor chain) instead of 50 fp32 threshold
    compares on replicated s. Exact given exact fp32 sqrt/divide.
  - Keys (bf16, exact small integers) bounce through DRAM and are replicated
    across 98 partitions by 2 DMAs per 32-row group (26x less replication
    bytes than fp32 s replication would need).
  - Staircase stair[j] = (k >= j), j=1..49 per element half, via in-place DVE
    is_ge (bf16, 2x DVE perf mode) inside a 32-row mega-tile.
  - mm1 is ONE bf16 matmul per row: contraction 107 = 49 stairs x 2 halves
    + x,y rows + 4 ego rows + ones row. Ego and the b1+T[0] constant ride the
    contraction (stride-0 broadcast DMAs), removing the per-row activation
    bias so relu can batch 2 PSUM banks per instruction.
  - relu on Act (bias-free), scores = bf16 mm2 (+-1 weights) into 4-row PSUM
    groups (tile_position col 0/32/64/96), b2 added during the PSUM->SBUF
    copy into a 32-row staging tile, 4 DMAs out per 32 rows.
"""

import numpy as np

N_CORES = 8
B, K = 2048, 1024
NB = 50
D = 64
BS = B // N_CORES  # 256 rows/core
F = K // 2  # 512; elements (c, c+512) of a row share a column
NTH = NB - 1  # 49 staircase rows per element half (j = 1..49)
KC = 2 * NTH + 4 + 5  # 107 = stairs A/B + xA,xB,yA,yB + 4 ego + ones
RPB = 64  # rows per block
RPG = 32  # rows per mega-tile group
GPB = RPB // RPG  # groups per block
RELU_DVE_EVERY = 2  # every Nth 2-row relu unit runs on DVE instead of Act
COPY_ACT_EVERY = 2  # every Nth score copy runs on Act instead of DVE
MEGA_BUFS = 2  # mega-tile double/triple buffering
EDGE_SPLIT = False  # split first/last block into 16-row mini-groups
XY_Q = "sync"  # queue for the x/y mega-row DMAs: sync | gpsimd | scalar
WORK_BUFS = 3
HR_BUFS = 8
SCR_BUFS = 3


def _consts(W1, b1, W2, b2):
    import ml_dtypes

    W1 = np.asarray(W1, np.float32)
    b1 = np.asarray(b1, np.float32)
    W2 = np.asarray(W2, np.float32)
    absw2 = np.abs(W2[:, 0])
    sgn = np.where(W2[:, 0] < 0, -1.0, 1.0).astype(np.float32)

    Tt = W1[:NB] * absw2[None, :]  # (50, 64)
    dT = (Tt[1:] - Tt[:-1]).astype(np.float32)  # (49, 64), rows j=1..49
    wx = W1[50] * absw2
    wy = W1[51] * absw2
    wego = W1[52:56] * absw2[None, :]  # (4, 64)
    wc = (b1 + W1[0]) * absw2  # b1 + T[0] folded

    bf = ml_dtypes.bfloat16
    smm1 = np.zeros((KC, 128), bf)
    smm1[0:NTH, 0:64] = dT.astype(bf)
    smm1[NTH : 2 * NTH, 64:128] = dT.astype(bf)
    smm1[98, 0:64] = wx.astype(bf)
    smm1[99, 64:128] = wx.astype(bf)
    smm1[100, 0:64] = wy.astype(bf)
    smm1[101, 64:128] = wy.astype(bf)
    for i in range(4):
        smm1[102 + i, 0:64] = wego[i].astype(bf)
        smm1[102 + i, 64:128] = wego[i].astype(bf)
    smm1[106, 0:64] = wc.astype(bf)
    smm1[106, 64:128] = wc.astype(bf)

    # smm2e: first mm2 of each PSUM group writes the FULL bank (cols 2:128
    # zero) so the later full-tile copy never reads uninitialized PSUM.
    smm2 = np.zeros((128, 2), bf)
    smm2[0:64, 0] = sgn.astype(bf)
    smm2[64:128, 1] = sgn.astype(bf)
    smm2e = np.zeros((128, 128), bf)
    smm2e[:, 0:2] = smm2

    thr = np.zeros((2 * NTH, 1), np.float32)
    thr[0:NTH, 0] = np.arange(1, NB, dtype=np.float32)
    thr[NTH : 2 * NTH, 0] = np.arange(1, NB, dtype=np.float32)

    b2col = np.full((128, 1), float(np.asarray(b2, np.float32).reshape(-1)[0]),
                    np.float32)
    return dict(smm1=smm1, smm2=smm2, smm2e=smm2e, thr=thr, b2col=b2col)


def _ego_rows(ego):
    """Stage ego (+ a ones row) replicated along columns: [B, 5, F] bf16."""
    import ml_dtypes

    e5 = np.ones((ego.shape[0], 5), np.float32)
    e5[:, 0:4] = ego
    e5b = e5.astype(ml_dtypes.bfloat16)
    return np.ascontiguousarray(np.broadcast_to(e5b[:, :, None],
                                                (ego.shape[0], 5, F)))


def _build():
    import concourse.bass as bass
    import concourse.mybir as mybir
    from concourse import bacc
    from concourse.tile import TileContext

    f32 = mybir.dt.float32
    bf16 = mybir.dt.bfloat16
    Alu = mybir.AluOpType
    Act = mybir.ActivationFunctionType

    nc = bacc.Bacc("TRN2", target_bir_lowering=False, debug=False,
                   num_devices=N_CORES)

    gpx_d = nc.declare_dram_parameter("gpx", [BS, K], f32, isOutput=False)
    gpy_d = nc.declare_dram_parameter("gpy", [BS, K], f32, isOutput=False)
    egor_d = nc.declare_dram_parameter("egor", [BS, 5, F], bf16, isOutput=False)
    smm1_d = nc.declare_dram_parameter("smm1", [KC, 128], bf16, isOutput=False)
    smm2_d = nc.declare_dram_parameter("smm2", [128, 2], bf16, isOutput=False)
    smm2e_d = nc.declare_dram_parameter("smm2e", [128, 128], bf16, isOutput=False)
    thr_d = nc.declare_dram_parameter("thr", [2 * NTH, 1], f32, isOutput=False)
    b2_d = nc.declare_dram_parameter("b2c", [128, 1], f32, isOutput=False)
    sc_d = nc.declare_dram_parameter("scores", [BS, K], f32, isOutput=True)

    with TileContext(nc) as tc:
        with (
            tc.tile_pool(name="consts", bufs=1) as cpool,
            tc.tile_pool(name="work", bufs=WORK_BUFS) as wpool,
            tc.tile_pool(name="mega", bufs=MEGA_BUFS) as spool,
            tc.tile_pool(name="mega16", bufs=2) as spool16,
            tc.tile_pool(name="hrelu", bufs=HR_BUFS) as hpool,
            tc.tile_pool(name="scr", bufs=SCR_BUFS) as scpool,
            tc.tile_pool(name="scr16", bufs=2) as scpool16,
            tc.tile_pool(name="edram", bufs=1, space="DRAM") as epool,
            tc.tile_pool(name="sdram", bufs=2, space="DRAM") as dpool,
            tc.tile_pool(name="p1", bufs=3, space="PSUM") as p1pool,
            tc.tile_pool(name="p2", bufs=2, space="PSUM") as p2pool,
        ):
            smm1_s = cpool.tile([KC, 128], bf16, tag="smm1")
            smm2_s = cpool.tile([128, 2], bf16, tag="smm2")
            smm2e_s = cpool.tile([128, 128], bf16, tag="smm2e")
            thr_s = cpool.tile([2 * NTH, 1], f32, tag="thr")
            b2_s = cpool.tile([128, 1], f32, tag="b2c")
            nc.sync.dma_start(out=smm1_s[:], in_=smm1_d[:])
            nc.sync.dma_start(out=smm2_s[:], in_=smm2_d[:])
            nc.sync.dma_start(out=smm2e_s[:], in_=smm2e_d[:])
            nc.sync.dma_start(out=thr_s[:], in_=thr_d[:])
            nc.sync.dma_start(out=b2_s[:], in_=b2_d[:])

            for blk in range(BS // RPB):
                r0 = blk * RPB
                gpxp = wpool.tile([128, F], f32, tag="gpxp")
                gpyp = wpool.tile([128, F], f32, tag="gpyp")
                nc.gpsimd.dma_start(
                    out=gpxp[:],
                    in_=gpx_d[r0 : r0 + RPB].rearrange("b (h k) -> (b h) k", h=2))
                nc.gpsimd.dma_start(
                    out=gpyp[:],
                    in_=gpy_d[r0 : r0 + RPB].rearrange("b (h k) -> (b h) k", h=2))

                xx = wpool.tile([128, F], f32, tag="xx")
                ss = wpool.tile([128, F], f32, tag="ss")
                nc.vector.tensor_tensor(out=xx[:], in0=gpxp[:], in1=gpxp[:],
                                        op=Alu.mult)
                nc.vector.tensor_tensor(out=ss[:], in0=gpyp[:], in1=gpyp[:],
                                        op=Alu.mult)
                nc.vector.tensor_tensor(out=ss[:], in0=ss[:], in1=xx[:],
                                        op=Alu.add)
                # d = sqrt(s); v = (d*(1/50))*50;
                # floor(v) = rne(v) - (rne(v) > v), rne via the +-1.5*2^23
                # magic constant (valid-ISA ops only: divide/mod are not
                # legal DVE tensor_scalar ops on hardware).
                MAGIC = 12582912.0  # 1.5 * 2^23
                dv = wpool.tile([128, F], f32, tag="dv")
                nc.scalar.activation(dv[:], ss[:], Act.Sqrt)
                nc.vector.tensor_scalar(out=dv[:], in0=dv[:], scalar1=0.02,
                                        scalar2=50.0, op0=Alu.mult,
                                        op1=Alu.mult)
                mm = wpool.tile([128, F], f32, tag="mm")
                nc.vector.tensor_scalar(out=mm[:], in0=dv[:], scalar1=MAGIC,
                                        scalar2=-MAGIC, op0=Alu.add,
                                        op1=Alu.add)
                gg = wpool.tile([128, F], f32, tag="gg")
                nc.vector.tensor_tensor(out=gg[:], in0=mm[:], in1=dv[:],
                                        op=Alu.is_gt)
                nc.vector.tensor_tensor(out=mm[:], in0=mm[:], in1=gg[:],
                                        op=Alu.subtract)
                d16 = wpool.tile([128, F], bf16, tag="d16")
                nc.vector.tensor_scalar(out=d16[:], in0=mm[:], scalar1=49.0,
                                        scalar2=None, op0=Alu.min)
                # bf16 copies of x, y for mm1 xy contraction rows
                gxb = wpool.tile([128, F], bf16, tag="gxb")
                gyb = wpool.tile([128, F], bf16, tag="gyb")
                nc.vector.tensor_copy(out=gxb[:], in_=gpxp[:])
                nc.vector.tensor_copy(out=gyb[:], in_=gpyp[:])
                # bounce keys + bf16 xy through DRAM for group-level broadcast
                d16_d = dpool.tile([RPB, K], bf16, tag="d16d")
                xb_d = dpool.tile([RPB, K], bf16, tag="xbd")
                yb_d = dpool.tile([RPB, K], bf16, tag="ybd")
                for sb, dr in ((d16, d16_d), (gxb, xb_d), (gyb, yb_d)):
                    nc.scalar.dma_start(
                        out=dr[:].rearrange("b (h k) -> (b h) k", h=2),
                        in_=sb[:])

                if EDGE_SPLIT and blk == 0:
                    plan = [(0, 16), (16, 16), (32, 32)]
                elif EDGE_SPLIT and blk == BS // RPB - 1:
                    plan = [(0, 32), (32, 16), (48, 16)]
                else:
                    plan = [(0, RPG), (RPG, RPG)]
                for g0, rpg in plan:
                    gr0 = r0 + g0
                    CW = rpg * F
                    if rpg == RPG:
                        mega = spool.tile([KC, CW], bf16, tag="mega")
                    else:
                        mega = spool16.tile([KC, CW], bf16, tag="mega16")

                    # key replication: 49 partitions x 32 rows, A and B halves,
                    # split into 8-row chunks so transfers interleave finely
                    HR = 8
                    NCH = rpg // HR
                    for ch in range(NCH):
                        for half, prow in ((0, 0), (1, NTH)):
                            src = d16_d[g0 + ch * HR : g0 + (ch + 1) * HR]
                            src_a = bass.AP(
                                tensor=src.tensor, offset=src.offset + half * F,
                                ap=[[0, NTH], [K, HR], [1, F]])
                            nc.sync.dma_start(
                                out=mega[prow : prow + NTH,
                                         ch * HR * F : (ch + 1) * HR * F
                                         ].rearrange("p (r c) -> p r c", r=HR),
                                in_=src_a)
                    # x,y rows from DRAM bf16 scratch
                    for dr, prow in ((xb_d, 98), (yb_d, 100)):
                        src = dr[g0 : g0 + rpg]
                        src_a = bass.AP(
                            tensor=src.tensor, offset=src.offset,
                            ap=[[F, 2], [K, rpg], [1, F]])
                        getattr(nc, XY_Q).dma_start(
                            out=mega[prow : prow + 2, :].rearrange(
                                "p (r c) -> p r c", r=rpg),
                            in_=src_a)
                    # ego rows + ones row (pre-replicated along columns)
                    esrc = egor_d[gr0 : gr0 + rpg]
                    esrc_a = bass.AP(tensor=esrc.tensor, offset=esrc.offset,
                                     ap=[[F, 5], [5 * F, rpg], [1, F]])
                    nc.gpsimd.dma_start(
                        out=mega[102:107, :].rearrange("p (r c) -> p r c", r=rpg),
                        in_=esrc_a)

                    # staircase in place: is_ge per 8-row chunk
                    for ci in range(NCH):
                        sl = slice(ci * CW // NCH, (ci + 1) * CW // NCH)
                        nc.vector.tensor_scalar(
                            out=mega[0 : 2 * NTH, sl],
                            in0=mega[0 : 2 * NTH, sl],
                            scalar1=thr_s[:], scalar2=None, op0=Alu.is_ge)

                    if rpg == RPG:
                        scr = scpool.tile([128, rpg // 4 * F], f32, tag="scr")
                    else:
                        scr = scpool16.tile([128, rpg // 4 * F], f32,
                                            tag="scr16")
                    for pj in range(rpg // 4):
                        rr = gr0 + pj * 4
                        j0 = pj * 4
                        p1a = p1pool.tile([128, 2 * F], f32, tag="p1")
                        p1b = p1pool.tile([128, 2 * F], f32, tag="p1")
                        for jj in range(4):
                            pt = p1a if jj < 2 else p1b
                            nc.tensor.matmul(
                                pt[:, (jj % 2) * F : (jj % 2 + 1) * F],
                                lhsT=smm1_s[:],
                                rhs=mega[:, (j0 + jj) * F : (j0 + jj + 1) * F],
                                start=True, stop=True)
                        hra = hpool.tile([128, 2 * F], bf16, tag="hr")
                        hrb = hpool.tile([128, 2 * F], bf16, tag="hr")
                        nc.scalar.activation(hra[:], p1a[:], Act.Relu)
                        if pj % RELU_DVE_EVERY == RELU_DVE_EVERY - 1:
                            nc.vector.tensor_scalar(
                                out=hrb[:], in0=p1b[:], scalar1=0.0,
                                scalar2=None, op0=Alu.max)
                        else:
                            nc.scalar.activation(hrb[:], p1b[:], Act.Relu)
                        p2 = p2pool.tile([128, F], f32, tag="p2")
                        nc.tensor.matmul(p2[:], lhsT=smm2e_s[:],
                                         rhs=hra[:, 0:F], start=True, stop=True)
                        for jj in range(1, 4):
                            hx = hra if jj < 2 else hrb
                            nc.tensor.matmul(
                                p2[32 * jj : 32 * jj + 2, :],
                                lhsT=smm2_s[:],
                                rhs=hx[:, (jj % 2) * F : (jj % 2 + 1) * F],
                                start=True, stop=True,
                                tile_position=(0, 32 * jj))
                        # b2 add during PSUM -> SBUF staging copy
                        dstc = scr[:, pj * F : (pj + 1) * F]
                        if pj % COPY_ACT_EVERY == 0:
                            nc.scalar.activation(dstc, p2[:], Act.Identity,
                                                 bias=b2_s[:, 0:1])
                        else:
                            nc.vector.tensor_scalar(
                                out=dstc, in0=p2[:], scalar1=b2_s[:, 0:1],
                                scalar2=None, op0=Alu.add)
                    # 4 DMAs out per 32 rows: partition pair {32a, 32a+1}
                    # holds rows gr0 + a + 4t for t in 0..8
                    for a in range(4):
                        dst = sc_d[gr0 + a : gr0 + a + 1]
                        dst_a = bass.AP(
                            tensor=dst.tensor, offset=dst.offset,
                            ap=[[F, 2], [4 * K, rpg // 4], [1, F]])
                        nc.gpsimd.dma_start(
                            out=dst_a,
                            in_=scr[32 * a : 32 * a + 2, :].rearrange(
                                "p (t c) -> p t c", t=rpg // 4))

    nc.finalize()
    return nc


_CACHE = {}


def make_in_maps(goal_positions, ego_state, W1, b1, W2, b2):
    gp = np.asarray(goal_positions, np.float32)
    gpx = np.ascontiguousarray(gp[..., 0])
    gpy = np.ascontiguousarray(gp[..., 1])
    egor = _ego_rows(np.asarray(ego_state, np.float32))
    c = _consts(W1, b1, W2, b2)
    in_maps = []
    for i in range(N_CORES):
        in_maps.append({
            "gpx": gpx[i * BS : (i + 1) * BS],
            "gpy": gpy[i * BS : (i + 1) * BS],
            "egor": egor[i * BS : (i + 1) * BS],
            "smm1": c["smm1"], "smm2": c["smm2"], "smm2e": c["smm2e"],
            "thr": c["thr"], "b2c": c["b2col"],
        })
    return in_maps


def kernel(goal_positions, ego_state, W1, b1, W2, b2):
    from concourse.bass_utils import run_bass_kernel_spmd

    if "nc" not in _CACHE:
        _CACHE["nc"] = _build()
    nc = _CACHE["nc"]

    in_maps = make_in_maps(goal_positions, ego_state, W1, b1, W2, b2)
    res = run_bass_kernel_spmd(nc, in_maps, core_ids=list(range(N_CORES)))
    out = np.concatenate([res.results[i]["scores"] for i in range(N_CORES)],
                         axis=0)
    return out.astype(np.float32)
